# revision 1
# baseline (speedup 1.0000x reference)
"""Trainium2 Bass kernel for nn_CausalRecurrenceLayer.

Sharding: 8 cores = 4 batches x 2 sequence-halves. Device layout is
channel-major [c, t] for the conv/gate matmuls and the hardware scan
(tensor_tensor_scan); the output projection is emitted as [t, j] so it DMAs
directly into the [B, L, d] output.

Pipeline per core (b = core//2, th = core%2):
  A: causal depthwise conv as 4 accumulating diagonal matmuls (PE, f32r)
     -> gates r,i via bf16 matmuls -> tanh/exp (one ACT table set)
     -> decay a (stored as a-1 in fp16, spilled to DRAM)
     -> gated input bb (spilled to DRAM) -> pass-1 scan (local h_last)
  AllGather h_last across sequence-half pairs (4 KB)
  B: true scan with received initial state -> output projection (PE, f32r)
     -> RMSNorm (Square-accumulate + sqrt + reciprocal) -> DMA out.

Self-contained: hardcodes shapes B=4, L=4096, d=1024.
"""
import sys

sys.path.insert(0, "/opt/trn_rl_repo")

import numpy as np
import ml_dtypes

import concourse.bass as bass  # noqa: F401
from concourse.bass import _add_dep_helper
import concourse.tile as tile
from concourse import bacc, mybir
from concourse import bass_utils

F32 = mybir.dt.float32
F32R = mybir.dt.float32r
F16 = mybir.dt.float16
BF16 = mybir.dt.bfloat16
AF = mybir.ActivationFunctionType
OP = mybir.AluOpType

B, L, D = 4, 4096, 1024
TH = L // 2      # per-core sequence extent
TT = 512         # time tile
NT = TH // TT    # 4
P = 128
CB = D // P      # 8 channel blocks
EPS = 1e-6

_compiled = {}


def _build():
    nc = bacc.Bacc("TRN2", target_bir_lowering=False, debug=False, num_devices=8)

    x_d = nc.dram_tensor("x_sh", [D, TH + 3], F32R, kind="ExternalInput").ap()
    dw_d = nc.dram_tensor("dwk", [D, 4 * P], F32R, kind="ExternalInput").ap()
    wr_d = nc.dram_tensor("wrT", [D, D], F16, kind="ExternalInput").ap()
    wi_d = nc.dram_tensor("wiT", [D, D], F16, kind="ExternalInput").ap()
    wo_d = nc.dram_tensor("woT", [D, D], F32R, kind="ExternalInput").ap()
    br_d = nc.dram_tensor("br_c", [P, CB], F32, kind="ExternalInput").ap()   # b_r/2
    bi_d = nc.dram_tensor("bi_c", [P, CB], F32, kind="ExternalInput").ap()   # b_i/2
    cb_d = nc.dram_tensor("cb_c", [P, CB], F32, kind="ExternalInput").ap()   # conv bias
    c1_d = nc.dram_tensor("c1_c", [P, CB], F32, kind="ExternalInput").ap()   # 4*ln(a_base)
    tm_d = nc.dram_tensor("tmask", [P, 1], F32, kind="ExternalInput").ap()
    y_d = nc.dram_tensor("y", [TH, D], F32, kind="ExternalOutput").ap()

    last_act = [None]
    _CHAINED = (AF.Tanh, AF.Exp, AF.Sqrt)

    def act(out, in_, func, **kw):
        ins = nc.scalar.activation(out, in_, func, **kw)
        if func in _CHAINED:
            if last_act[0] is not None:
                _add_dep_helper(ins.ins, last_act[0].ins, reason="act table order")
            last_act[0] = ins
        return ins

    with tile.TileContext(nc) as tc:
        with (
            tc.tile_pool(name="wpool", bufs=1) as wpool,
            tc.tile_pool(name="sbuf", bufs=1) as sb,
            tc.tile_pool(name="store", bufs=1) as store,
            tc.tile_pool(name="psum", bufs=1, space="PSUM") as ps,
            tc.tile_pool(name="dram", bufs=1, space="DRAM") as dp,
        ):
            # ---- resident weights / constants ----
            br_t = wpool.tile([P, CB], F32, tag="br")
            nc.scalar.dma_start(br_t[:], br_d)
            bi_t = wpool.tile([P, CB], F32, tag="bi")
            nc.scalar.dma_start(bi_t[:], bi_d)
            cb_t = wpool.tile([P, CB], F32, tag="cbias")
            nc.scalar.dma_start(cb_t[:], cb_d)
            c1_t = wpool.tile([P, CB], F32, tag="c1")
            nc.scalar.dma_start(c1_t[:], c1_d)
            tm_t = wpool.tile([P, 1], F32, tag="tm")
            nc.scalar.dma_start(tm_t[:], tm_d)
            wr_t, wi_t, wo_t, dw_t = [], [], [], []
            for cb in range(CB):
                t = wpool.tile([P, 4 * P], F32R, tag=f"dw{cb}", name=f"dw{cb}")
                nc.sync.dma_start(t[:], dw_d[cb * P:(cb + 1) * P, :])
                dw_t.append(t)
            for cb in range(CB):
                t = wpool.tile([P, D], F16, tag=f"wr{cb}", name=f"wr{cb}")
                nc.sync.dma_start(t[:], wr_d[cb * P:(cb + 1) * P, :])
                wr_t.append(t)
                t = wpool.tile([P, D], F16, tag=f"wi{cb}", name=f"wi{cb}")
                nc.sync.dma_start(t[:], wi_d[cb * P:(cb + 1) * P, :])
                wi_t.append(t)
            eps_t = wpool.tile([P, 1], F32, tag="eps")
            nc.vector.memset(eps_t[:], EPS)
            zeros_t = wpool.tile([P, TT], F32, tag="zeros")
            nc.vector.memset(zeros_t[:], 0.0)
            for cb in range(CB):
                t = wpool.tile([P, D], F32R, tag=f"wo{cb}", name=f"wo{cb}")
                nc.sync.dma_start(t[:], wo_d[cb * P:(cb + 1) * P, :])
                wo_t.append(t)

            hl_sb = store.tile([P, CB], F32, tag="hl")
            s1_spill = dp.tile([D, TH], F32, tag="s1sp")
            p_spill = dp.tile([D, TH], F16, tag="psp")
            ag_in = dp.tile([1, D], F32, tag="ag_in")
            ag_out = dp.tile([2, D], F32, tag="ag_out")

            # =========== PHASE A ===========
            scan1_prev = [None] * CB
            pscan_prev = [None] * CB
            for t0 in range(NT):
                # -- conv on PE: xc = sum_k diag(w_k) @ x[:, t+k-3] + bias --
                xc_t = []
                xcb_t = []
                for cb in range(CB):
                    xt = sb.tile([P, TT + 3], F32R, tag="xraw", bufs=2)
                    nc.scalar.dma_start(xt[:], x_d[cb * P:(cb + 1) * P, t0 * TT:t0 * TT + TT + 3])
                    xc_ps = ps.tile([P, TT], F32, tag="xc_ps", bufs=2)
                    for k in range(4):
                        nc.tensor.matmul(xc_ps[:], dw_t[cb][:, k * P:(k + 1) * P],
                                         xt[:, k:k + TT], start=(k == 0), stop=(k == 3))
                    xc = sb.tile([P, TT], F16, tag="xc", bufs=16)
                    act(xc[:], xc_ps[:], AF.Identity, bias=cb_t[:, cb:cb + 1])
                    xc_t.append(xc)
                    xcb_t.append(xc)

                # -- gate matmuls + tanh/exp batch (exp_and_others set) --
                th_i_t = []
                am1_tiles = [None] * CB
                for cb in range(CB):
                    r_ps = ps.tile([P, TT], F32, tag="r_ps", bufs=2)
                    i_ps = ps.tile([P, TT], F32, tag="i_ps", bufs=2)
                    for kb in range(CB):
                        nc.tensor.matmul(r_ps[:], wr_t[kb][:, cb * P:(cb + 1) * P],
                                         xcb_t[kb][:], start=(kb == 0), stop=(kb == CB - 1))
                    for kb in range(CB):
                        nc.tensor.matmul(i_ps[:], wi_t[kb][:, cb * P:(cb + 1) * P],
                                         xcb_t[kb][:], start=(kb == 0), stop=(kb == CB - 1))
                    th_r = sb.tile([P, TT], F32, tag="th_r", bufs=2)
                    act(th_r[:], r_ps[:], AF.Tanh, bias=br_t[:, cb:cb + 1], scale=0.5)
                    a_t = sb.tile([P, TT], F32, tag="a_t", bufs=2)
                    act(a_t[:], th_r[:], AF.Exp,
                        bias=c1_t[:, cb:cb + 1], scale=c1_t[:, cb:cb + 1])
                    am1 = sb.tile([P, TT], F16, tag="am1", bufs=10, name=f"am1_{cb}_{t0}")
                    nc.vector.tensor_scalar_add(am1[:], a_t[:], -1.0)
                    am1_tiles[cb] = am1
                    th_i = sb.tile([P, TT], F16, tag="th_i", bufs=8)
                    act(th_i[:], i_ps[:], AF.Tanh, bias=bi_t[:, cb:cb + 1], scale=0.5)
                    th_i_t.append(th_i)

                # -- sqrt batch + gated input + pass-1 scan --
                for cb in range(CB):
                    am1_sl = am1_tiles[cb][:]
                    ap1 = sb.tile([P, TT], F32, tag="ap1", bufs=2)
                    nc.vector.tensor_scalar_add(ap1[:], am1_sl, 1.0)
                    w = sb.tile([P, TT], F32, tag="w_t", bufs=2)
                    nc.vector.tensor_tensor(w[:], ap1[:], ap1[:], OP.mult)
                    scl = sb.tile([P, TT], F32, tag="scl", bufs=2)
                    act(scl[:], w[:], AF.Sqrt, scale=-1.0, bias=1.0)
                    u = sb.tile([P, TT], F32, tag="u_t", bufs=2)
                    nc.vector.tensor_scalar(u[:], th_i_t[cb][:], 0.5, 0.5, OP.mult, OP.add)
                    b1 = sb.tile([P, TT], F32, tag="b1", bufs=2)
                    nc.vector.tensor_tensor(b1[:], u[:], scl[:], OP.mult)
                    bb = sb.tile([P, TT], F32, tag="bb", bufs=2)
                    nc.vector.tensor_tensor(bb[:], b1[:], xc_t[cb][:], OP.mult)
                    s1 = sb.tile([P, TT], F32, tag="s1", bufs=2)
                    init = 0.0 if t0 == 0 else scan1_prev[cb][:, 0:1]
                    nc.vector.tensor_tensor_scan(s1[:], ap1[:], bb[:], init, OP.mult, OP.add)
                    nc.sync.dma_start(s1_spill[cb * P:(cb + 1) * P, t0 * TT:(t0 + 1) * TT], s1[:])
                    pp = sb.tile([P, TT], F16, tag="pp", bufs=2)
                    pinit = 1.0 if t0 == 0 else pscan_prev[cb][:, 0:1]
                    nc.vector.tensor_tensor_scan(pp[:], ap1[:], zeros_t[:], pinit, OP.mult, OP.add)
                    nc.sync.dma_start(p_spill[cb * P:(cb + 1) * P, t0 * TT:(t0 + 1) * TT], pp[:])
                    if t0 == NT - 1:
                        nc.vector.tensor_copy(hl_sb[:, cb:cb + 1], s1[:, TT - 1:TT])
                    else:
                        cy = sb.tile([P, 1], F32, tag=f"cy{cb}", name=f"cy{cb}", bufs=2)
                        nc.vector.tensor_copy(cy[:], s1[:, TT - 1:TT])
                        scan1_prev[cb] = cy
                        py = sb.tile([P, 1], F32, tag=f"py{cb}", name=f"py{cb}", bufs=2)
                        nc.vector.tensor_copy(py[:], pp[:, TT - 1:TT])
                        pscan_prev[cb] = py

            # =========== collective: exchange local h_last ===========
            nc.sync.dma_start(ag_in[:].rearrange("one (cb p) -> p (one cb)", p=P), hl_sb[:])
            nc.gpsimd.collective_compute(
                "AllGather", OP.bypass,
                replica_groups=[[0, 1], [2, 3], [4, 5], [6, 7]],
                ins=[ag_in[:].opt()], outs=[ag_out[:].opt()],
            )
            g0 = store.tile([P, CB], F32, tag="g0")
            nc.sync.dma_start(g0[:], ag_out[0:1, :].rearrange("one (cb p) -> p (one cb)", p=P))
            init_c = store.tile([P, CB], F32, tag="init_c")
            nc.vector.tensor_scalar_mul(init_c[:], g0[:], tm_t[:, 0:1])

            # =========== PHASE B: true scan + out-proj + RMSNorm ===========
            for t0 in range(NT):
                h_t = []
                for cb in range(CB):
                    s1r = sb.tile([P, TT], F32, tag="s1r", bufs=4)
                    nc.scalar.dma_start(s1r[:], s1_spill[cb * P:(cb + 1) * P, t0 * TT:(t0 + 1) * TT])
                    ppr = sb.tile([P, TT], F16, tag="ppr", bufs=6)
                    nc.sync.dma_start(ppr[:], p_spill[cb * P:(cb + 1) * P, t0 * TT:(t0 + 1) * TT])
                    h = sb.tile([P, TT], F32R, tag="h", bufs=9)
                    nc.vector.scalar_tensor_tensor(h[:], ppr[:], init_c[:, cb:cb + 1],
                                                   s1r[:], OP.mult, OP.add)
                    h_t.append(h)
                for ch in range(TT // P):
                    o_ps = ps.tile([P, D], F32, tag="o_ps", bufs=1)
                    for jh in range(2):
                        for kb in range(CB):
                            nc.tensor.matmul(
                                o_ps[:, jh * 512:(jh + 1) * 512],
                                h_t[kb][:, ch * P:(ch + 1) * P],
                                wo_t[kb][:, jh * 512:(jh + 1) * 512],
                                start=(kb == 0), stop=(kb == CB - 1))
                    sq0 = sb.tile([P, 512], F32, tag="sq0", bufs=1)
                    ss0 = sb.tile([P, 1], F32, tag="ss0", bufs=2)
                    act(sq0[:], o_ps[:, 0:512], AF.Square, accum_out=ss0[:])
                    sq1 = sb.tile([P, 512], F32, tag="sq1", bufs=1)
                    ss1 = sb.tile([P, 1], F32, tag="ss1", bufs=2)
                    act(sq1[:], o_ps[:, 512:1024], AF.Square, accum_out=ss1[:])
                    ssum = sb.tile([P, 1], F32, tag="ssum", bufs=2)
                    nc.vector.tensor_tensor(ssum[:], ss0[:], ss1[:], OP.add)
                    s = sb.tile([P, 1], F32, tag="s_rms", bufs=2)
                    act(s[:], ssum[:], AF.Sqrt, scale=1.0 / D, bias=eps_t[:, 0:1])
                    rinv = sb.tile([P, 1], F32, tag="rinv", bufs=2)
                    nc.vector.reciprocal(rinv[:], s[:])
                    y_sb = sb.tile([P, D], F32, tag="y_sb", bufs=2)
                    nc.vector.tensor_scalar_mul(y_sb[:, 0:512], o_ps[:, 0:512], rinv[:, 0:1])
                    nc.vector.tensor_scalar_mul(y_sb[:, 512:1024], o_ps[:, 512:1024], rinv[:, 0:1])
                    nc.sync.dma_start(y_d[t0 * TT + ch * P: t0 * TT + (ch + 1) * P, :], y_sb[:])

    nc.compile()
    return nc


def kernel(**inputs):
    x = np.asarray(inputs["x"], np.float32)
    conv_w = np.asarray(inputs["conv_w"], np.float32)
    conv_b = np.asarray(inputs["conv_b"], np.float32)
    W_r = np.asarray(inputs["W_r"], np.float32)
    b_r = np.asarray(inputs["b_r"], np.float32)
    W_i = np.asarray(inputs["W_i"], np.float32)
    b_i = np.asarray(inputs["b_i"], np.float32)
    log_a = np.asarray(inputs["log_a"], np.float32)
    W_out = np.asarray(inputs["W_out"], np.float32)
    gamma = np.asarray(inputs["gamma"], np.float32)
    assert x.shape == (B, L, D), x.shape

    if "nc" not in _compiled:
        _compiled["nc"] = _build()
    nc = _compiled["nc"]

    def col(v):
        return np.ascontiguousarray(v.reshape(CB, P).T).astype(np.float32)

    xT = np.ascontiguousarray(x.transpose(0, 2, 1))            # [B, D, L]
    wrT = np.ascontiguousarray(W_r.T).astype(np.float16)
    wiT = np.ascontiguousarray(W_i.T).astype(np.float16)
    woT = np.ascontiguousarray((W_out * gamma[:, None]).T).astype(np.float32)
    # diagonal conv-tap blocks: dwk[cb*128+p, k*128+p] = conv_w[cb*128+p, 0, k]
    dwk = np.zeros((CB, P, 4, P), np.float32)
    idx = np.arange(P)
    for cb in range(CB):
        for k in range(4):
            dwk[cb, idx, k, idx] = conv_w[cb * P + idx, 0, k]
    dwk = dwk.reshape(D, 4 * P)
    a_base = 1.0 / (1.0 + np.exp(-log_a.astype(np.float64)))
    c1 = (8.0 * np.log(a_base)).astype(np.float32)
    common = {
        "wrT": wrT, "wiT": wiT, "woT": woT, "dwk": dwk,
        "br_c": col(0.5 * b_r), "bi_c": col(0.5 * b_i), "cb_c": col(conv_b),
        "c1_c": col(0.5 * c1),
    }
    in_maps = []
    for k in range(8):
        b, th = k // 2, k % 2
        xs = np.zeros((D, TH + 3), np.float32)
        lo = th * TH - 3
        if lo < 0:
            xs[:, 3:] = xT[b, :, 0:TH]
        else:
            xs[:] = xT[b, :, lo:lo + TH + 3]
        m = dict(common)
        m["x_sh"] = xs
        m["tmask"] = np.full((P, 1), float(th), np.float32)
        in_maps.append(m)

    import os
    trace = bool(int(os.environ.get("KERNEL_TRACE", "0")))
    kw = {}
    if trace:
        kw = dict(trace=True, trace_cores=list(range(8)))
    res = bass_utils.run_bass_kernel_spmd(nc, in_maps, core_ids=list(range(8)), **kw)
    _compiled["last_exec_time_ns"] = res.exec_time_ns
    _compiled["last_res"] = res

    out = np.empty((B, L, D), np.float32)
    for k in range(8):
        b, th = k // 2, k % 2
        out[b, th * TH:(th + 1) * TH, :] = res.results[k]["y"]
    return out



# revision 6
# speedup vs baseline: 1.6042x; 1.6042x over previous
"""Trainium2 Bass kernel for nn_CausalRecurrenceLayer.

Sharding: 8 cores = 4 batches x 2 sequence-halves. Device layout is
channel-major [c, t] for the conv/gate matmuls and the hardware scan
(tensor_tensor_scan); the output projection is emitted as [t, j] so it DMAs
directly into the [B, L, d] output.

v2: no DRAM spills (s1/pp resident in SBUF as f16 — scan state is fp32
internally so f16 output only costs output rounding), all matmuls f16
(conv taps, gates, out-proj), single 1024-wide out-proj matmuls into a
shared double-buffered PSUM pool, contiguous [P, CB] collective buffers.

Pipeline per core (b = core//2, th = core%2):
  A: causal depthwise conv as 4 accumulating diagonal matmuls (PE, f16)
     -> gates r,i via f16 matmuls -> tanh/exp (one ACT table set)
     -> a (f32) -> am1 = a-1 (f16) -> scale = sqrt(-am1*(am1+2)) (sqrt set)
     -> gated input bb -> s1 scan + pp cumprod scan (both f16, SBUF-resident)
  AllGather h_last across sequence-half pairs (4 KB)
  B: h = pp*h0 + s1 -> f16 out-proj (8x [128,1024] matmuls) -> RMSNorm
     (Square-accumulate + sqrt + reciprocal) -> DMA out.

Self-contained: hardcodes shapes B=4, L=4096, d=1024.
"""
import sys

sys.path.insert(0, "/opt/trn_rl_repo")

import numpy as np
import ml_dtypes

import concourse.bass as bass  # noqa: F401
from concourse.bass import _add_dep_helper
import concourse.tile as tile
from concourse import bacc, mybir
from concourse import bass_utils

F32 = mybir.dt.float32
F16 = mybir.dt.float16
BF16 = mybir.dt.bfloat16
AF = mybir.ActivationFunctionType
OP = mybir.AluOpType

B, L, D = 4, 4096, 1024
TH = L // 2      # per-core sequence extent
TT = 512         # time tile
NT = TH // TT    # 4
P = 128
CB = D // P      # 8 channel blocks
EPS = 1e-6

_compiled = {}


def _build():
    nc = bacc.Bacc("TRN2", target_bir_lowering=False, debug=False, num_devices=8)

    x_d = nc.dram_tensor("x_sh", [D, TH + 3], F16, kind="ExternalInput").ap()
    dw_d = nc.dram_tensor("dwk", [D, 4 * P], F16, kind="ExternalInput").ap()
    wr_d = nc.dram_tensor("wrT", [D, D], F16, kind="ExternalInput").ap()
    wi_d = nc.dram_tensor("wiT", [D, D], F16, kind="ExternalInput").ap()
    wo_d = nc.dram_tensor("woT", [D, D], F16, kind="ExternalInput").ap()
    br_d = nc.dram_tensor("br_c", [P, CB], F32, kind="ExternalInput").ap()   # b_r/2
    bi_d = nc.dram_tensor("bi_c", [P, CB], F32, kind="ExternalInput").ap()   # b_i/2
    cb_d = nc.dram_tensor("cb_c", [P, CB], F32, kind="ExternalInput").ap()   # conv bias
    c1_d = nc.dram_tensor("c1_c", [P, CB], F32, kind="ExternalInput").ap()   # 4*ln(a_base)
    tm_d = nc.dram_tensor("tmask", [P, 1], F32, kind="ExternalInput").ap()
    y_d = nc.dram_tensor("y", [TH, D], F32, kind="ExternalOutput").ap()

    last_act = [None]
    _CHAINED = (AF.Tanh, AF.Exp, AF.Sqrt)

    def act(out, in_, func, **kw):
        ins = nc.scalar.activation(out, in_, func, **kw)
        if func in _CHAINED:
            if last_act[0] is not None:
                _add_dep_helper(ins.ins, last_act[0].ins, reason="act table order")
            last_act[0] = ins
        return ins

    with tile.TileContext(nc) as tc:
        with (
            tc.tile_pool(name="wpool", bufs=1) as wpool,
            tc.tile_pool(name="sbuf", bufs=1) as sb,
            tc.tile_pool(name="store", bufs=1) as store,
            tc.tile_pool(name="psum", bufs=1, space="PSUM") as ps,
            tc.tile_pool(name="dram", bufs=1, space="DRAM") as dp,
        ):
            # ---- resident weights / constants ----
            br_t = wpool.tile([P, CB], F32, tag="br")
            nc.scalar.dma_start(br_t[:], br_d)
            bi_t = wpool.tile([P, CB], F32, tag="bi")
            nc.scalar.dma_start(bi_t[:], bi_d)
            cb_t = wpool.tile([P, CB], F32, tag="cbias")
            nc.scalar.dma_start(cb_t[:], cb_d)
            c1_t = wpool.tile([P, CB], F32, tag="c1")
            nc.scalar.dma_start(c1_t[:], c1_d)
            tm_t = wpool.tile([P, 1], F32, tag="tm")
            nc.scalar.dma_start(tm_t[:], tm_d)
            wr_t, wi_t, wo_t, dw_t = [], [], [], []
            for cb in range(CB):
                t = wpool.tile([P, 4 * P], F16, tag=f"dw{cb}", name=f"dw{cb}")
                nc.sync.dma_start(t[:], dw_d[cb * P:(cb + 1) * P, :])
                dw_t.append(t)
            for cb in range(CB):
                t = wpool.tile([P, D], F16, tag=f"wr{cb}", name=f"wr{cb}")
                nc.sync.dma_start(t[:], wr_d[cb * P:(cb + 1) * P, :])
                wr_t.append(t)
                t = wpool.tile([P, D], F16, tag=f"wi{cb}", name=f"wi{cb}")
                nc.sync.dma_start(t[:], wi_d[cb * P:(cb + 1) * P, :])
                wi_t.append(t)
            eps_t = wpool.tile([P, 1], F32, tag="eps")
            nc.vector.memset(eps_t[:], EPS)
            zeros_t = wpool.tile([P, TT], F16, tag="zeros")
            nc.vector.memset(zeros_t[:], 0.0)
            for cb in range(CB):
                t = wpool.tile([P, D], F16, tag=f"wo{cb}", name=f"wo{cb}")
                nc.sync.dma_start(t[:], wo_d[cb * P:(cb + 1) * P, :])
                wo_t.append(t)

            # resident scan outputs (f16; scan state is fp32 internally)
            s1_t = []
            pp_t = []
            for cb in range(CB):
                s1_t.append(store.tile([P, TH], F16, tag=f"s1_{cb}", name=f"s1_{cb}"))
                pp_t.append(store.tile([P, TH], F16, tag=f"pp_{cb}", name=f"pp_{cb}"))
            hl_sb = store.tile([P, CB], F32, tag="hl")
            ag_in = dp.tile([1, D], F32, tag="ag_in")
            ag_out = dp.tile([2, D], F32, tag="ag_out")

            # =========== PHASE A ===========
            scan1_prev = [None] * CB
            pscan_prev = [None] * CB
            for t0 in range(NT):
                # -- conv on PE: xc = sum_k diag(w_k) @ x[:, t+k-3] + bias --
                xc_t = []
                for cb in range(CB):
                    xt = sb.tile([P, TT + 3], F16, tag="xraw", bufs=3)
                    nc.scalar.dma_start(xt[:], x_d[cb * P:(cb + 1) * P, t0 * TT:t0 * TT + TT + 3])
                    xc_ps = ps.tile([P, TT], F32, tag="xc_ps", bufs=2)
                    for k in range(4):
                        nc.tensor.matmul(xc_ps[:], dw_t[cb][:, k * P:(k + 1) * P],
                                         xt[:, k:k + TT], start=(k == 0), stop=(k == 3))
                    xc = sb.tile([P, TT], F16, tag="xc", bufs=10)
                    act(xc[:], xc_ps[:], AF.Identity, bias=cb_t[:, cb:cb + 1])
                    xc_t.append(xc)

                # -- gate matmuls + tanh/exp batch (exp_and_others set) --
                th_i_t = []
                am1_tiles = [None] * CB
                for cb in range(CB):
                    ri_ps = ps.tile([P, 2 * TT], F32, tag="big", bufs=3)
                    r_ps = ri_ps[:, 0:TT]
                    i_ps = ri_ps[:, TT:2 * TT]
                    for kb in range(CB):
                        nc.tensor.matmul(r_ps, wr_t[kb][:, cb * P:(cb + 1) * P],
                                         xc_t[kb][:], start=(kb == 0), stop=(kb == CB - 1))
                    for kb in range(CB):
                        nc.tensor.matmul(i_ps, wi_t[kb][:, cb * P:(cb + 1) * P],
                                         xc_t[kb][:], start=(kb == 0), stop=(kb == CB - 1))
                    th_r = sb.tile([P, TT], F32, tag="th_r", bufs=2)
                    act(th_r[:], r_ps, AF.Tanh, bias=br_t[:, cb:cb + 1], scale=0.5)
                    a_t = sb.tile([P, TT], F32, tag="a_t", bufs=2)
                    act(a_t[:], th_r[:], AF.Exp,
                        bias=c1_t[:, cb:cb + 1], scale=c1_t[:, cb:cb + 1])
                    am1 = sb.tile([P, TT], F16, tag="am1", bufs=10, name=f"am1_{cb}_{t0}")
                    nc.vector.tensor_scalar_add(am1[:], a_t[:], -1.0)
                    am1_tiles[cb] = am1
                    th_i = sb.tile([P, TT], F16, tag="th_i", bufs=9)
                    act(th_i[:], i_ps, AF.Tanh, bias=bi_t[:, cb:cb + 1], scale=0.5)
                    th_i_t.append(th_i)

                # -- sqrt batch + gated input + scans (s1 + pp, SBUF-resident) --
                for cb in range(CB):
                    am1 = am1_tiles[cb]
                    ap1 = sb.tile([P, TT], F32, tag="ap1", bufs=2)
                    nc.vector.tensor_scalar_add(ap1[:], am1[:], 1.0)
                    t2 = sb.tile([P, TT], F16, tag="t2", bufs=2)
                    nc.vector.tensor_scalar_add(t2[:], am1[:], 2.0)
                    wn = sb.tile([P, TT], F16, tag="wn", bufs=2)
                    nc.vector.tensor_tensor(wn[:], am1[:], t2[:], OP.mult)
                    scl = sb.tile([P, TT], F16, tag="scl", bufs=2)
                    act(scl[:], wn[:], AF.Sqrt, scale=-1.0)
                    u = sb.tile([P, TT], F16, tag="u_t", bufs=2)
                    nc.vector.tensor_scalar(u[:], th_i_t[cb][:], 0.5, 0.5, OP.mult, OP.add)
                    b1 = sb.tile([P, TT], F16, tag="b1", bufs=2)
                    nc.vector.tensor_tensor(b1[:], u[:], scl[:], OP.mult)
                    bb = sb.tile([P, TT], F16, tag="bb", bufs=2)
                    nc.vector.tensor_tensor(bb[:], b1[:], xc_t[cb][:], OP.mult)
                    s1_sl = s1_t[cb][:, t0 * TT:(t0 + 1) * TT]
                    init = 0.0 if t0 == 0 else scan1_prev[cb][:, 0:1]
                    nc.vector.tensor_tensor_scan(s1_sl, ap1[:], bb[:], init, OP.mult, OP.add)
                    pp_sl = pp_t[cb][:, t0 * TT:(t0 + 1) * TT]
                    pinit = 1.0 if t0 == 0 else pscan_prev[cb][:, 0:1]
                    nc.vector.tensor_tensor_scan(pp_sl, ap1[:], zeros_t[:], pinit, OP.mult, OP.add)
                    if t0 == NT - 1:
                        nc.vector.tensor_copy(hl_sb[:, cb:cb + 1], s1_sl[:, TT - 1:TT])
                    else:
                        cy = sb.tile([P, 1], F32, tag=f"cy{cb}", name=f"cy{cb}", bufs=2)
                        nc.vector.tensor_copy(cy[:], s1_sl[:, TT - 1:TT])
                        scan1_prev[cb] = cy
                        py = sb.tile([P, 1], F32, tag=f"py{cb}", name=f"py{cb}", bufs=2)
                        nc.vector.tensor_copy(py[:], pp_sl[:, TT - 1:TT])
                        pscan_prev[cb] = py

            # =========== collective: exchange local h_last ===========
            # ag buffers laid out partition-major: linear = p*CB + cb
            nc.sync.dma_start(ag_in[:].rearrange("one (p cb) -> p (one cb)", p=P), hl_sb[:])
            nc.gpsimd.collective_compute(
                "AllGather", OP.bypass,
                replica_groups=[[0, 1], [2, 3], [4, 5], [6, 7]],
                ins=[ag_in[:].opt()], outs=[ag_out[:].opt()],
            )
            g0 = store.tile([P, CB], F32, tag="g0")
            nc.sync.dma_start(g0[:], ag_out[0:1, :].rearrange("one (p cb) -> p (one cb)", p=P))
            init_c = store.tile([P, CB], F32, tag="init_c")
            nc.vector.tensor_scalar_mul(init_c[:], g0[:], tm_t[:, 0:1])

            # =========== PHASE B: recombine + out-proj + RMSNorm ===========
            for t0 in range(NT):
                h_t = []
                for cb in range(CB):
                    h = sb.tile([P, TT], F16, tag="h", bufs=10)
                    nc.vector.scalar_tensor_tensor(
                        h[:], pp_t[cb][:, t0 * TT:(t0 + 1) * TT], init_c[:, cb:cb + 1],
                        s1_t[cb][:, t0 * TT:(t0 + 1) * TT], OP.mult, OP.add)
                    h_t.append(h)
                for ch in range(TT // P):
                    o_ps = ps.tile([P, 2 * TT], F32, tag="big", bufs=3)
                    for jh in range(2):
                        for kb in range(CB):
                            nc.tensor.matmul(
                                o_ps[:, jh * 512:(jh + 1) * 512],
                                h_t[kb][:, ch * P:(ch + 1) * P],
                                wo_t[kb][:, jh * 512:(jh + 1) * 512],
                                start=(kb == 0), stop=(kb == CB - 1))
                    sq = sb.tile([P, D], F32, tag="sq", bufs=2)
                    ssum = sb.tile([P, 1], F32, tag="ssum", bufs=2)
                    act(sq[:], o_ps[:], AF.Square, accum_out=ssum[:])
                    s = sb.tile([P, 1], F32, tag="s_rms", bufs=2)
                    act(s[:], ssum[:], AF.Sqrt, scale=1.0 / D, bias=eps_t[:, 0:1])
                    rinv = sb.tile([P, 1], F32, tag="rinv", bufs=2)
                    nc.vector.reciprocal(rinv[:], s[:])
                    y_sb = sb.tile([P, D], F32, tag="y_sb", bufs=2)
                    nc.vector.tensor_scalar_mul(y_sb[:, 0:512], o_ps[:, 0:512], rinv[:, 0:1])
                    nc.vector.tensor_scalar_mul(y_sb[:, 512:1024], o_ps[:, 512:1024], rinv[:, 0:1])
                    nc.sync.dma_start(y_d[t0 * TT + ch * P: t0 * TT + (ch + 1) * P, :], y_sb[:])

    nc.compile()
    return nc


def kernel(**inputs):
    x = np.asarray(inputs["x"], np.float32)
    conv_w = np.asarray(inputs["conv_w"], np.float32)
    conv_b = np.asarray(inputs["conv_b"], np.float32)
    W_r = np.asarray(inputs["W_r"], np.float32)
    b_r = np.asarray(inputs["b_r"], np.float32)
    W_i = np.asarray(inputs["W_i"], np.float32)
    b_i = np.asarray(inputs["b_i"], np.float32)
    log_a = np.asarray(inputs["log_a"], np.float32)
    W_out = np.asarray(inputs["W_out"], np.float32)
    gamma = np.asarray(inputs["gamma"], np.float32)
    assert x.shape == (B, L, D), x.shape

    if "nc" not in _compiled:
        _compiled["nc"] = _build()
    nc = _compiled["nc"]

    def col(v):
        return np.ascontiguousarray(v.reshape(CB, P).T).astype(np.float32)

    xT = np.ascontiguousarray(x.transpose(0, 2, 1))            # [B, D, L]
    wrT = np.ascontiguousarray(W_r.T).astype(np.float16)
    wiT = np.ascontiguousarray(W_i.T).astype(np.float16)
    woT = np.ascontiguousarray((W_out * gamma[:, None]).T).astype(np.float16)
    # diagonal conv-tap blocks: dwk[cb*128+p, k*128+p] = conv_w[cb*128+p, 0, k]
    dwk = np.zeros((CB, P, 4, P), np.float32)
    idx = np.arange(P)
    for cb in range(CB):
        for k in range(4):
            dwk[cb, idx, k, idx] = conv_w[cb * P + idx, 0, k]
    dwk = dwk.reshape(D, 4 * P).astype(np.float16)
    a_base = 1.0 / (1.0 + np.exp(-log_a.astype(np.float64)))
    c1 = (8.0 * np.log(a_base)).astype(np.float32)
    common = {
        "wrT": wrT, "wiT": wiT, "woT": woT, "dwk": dwk,
        "br_c": col(0.5 * b_r), "bi_c": col(0.5 * b_i), "cb_c": col(conv_b),
        "c1_c": col(0.5 * c1),
    }
    in_maps = []
    for k in range(8):
        b, th = k // 2, k % 2
        xs = np.zeros((D, TH + 3), np.float16)
        lo = th * TH - 3
        if lo < 0:
            xs[:, 3:] = xT[b, :, 0:TH].astype(np.float16)
        else:
            xs[:] = xT[b, :, lo:lo + TH + 3].astype(np.float16)
        m = dict(common)
        m["x_sh"] = xs
        m["tmask"] = np.full((P, 1), float(th), np.float32)
        in_maps.append(m)

    import os
    trace = bool(int(os.environ.get("KERNEL_TRACE", "0")))
    kw = {}
    if trace:
        kw = dict(trace=True, trace_cores=list(range(8)))
    res = bass_utils.run_bass_kernel_spmd(nc, in_maps, core_ids=list(range(8)), **kw)
    _compiled["last_exec_time_ns"] = res.exec_time_ns
    _compiled["last_res"] = res

    out = np.empty((B, L, D), np.float32)
    for k in range(8):
        b, th = k // 2, k % 2
        out[b, th * TH:(th + 1) * TH, :] = res.results[k]["y"]
    return out
